# revision 1
# baseline (speedup 1.0000x reference)
"""AdditiveAttention (Bahdanau) distributed Bass kernel for 8 TRN2 NeuronCores.

Computation (per batch b):
    qc[b,:]   = query[b] @ Wq + bq + bv                       # [512]
    z[b,s,:]  = value[b,s] @ Wv + qc[b]                       # pre-tanh
    score     = tanh(z) @ Wo          (+bo dropped: cancels in softmax)
    align     = softmax(score)        (no max-sub: |score| <= ~23, exp fits f32)
    out[b,:]  = align @ value[b]

Sharding: data-parallel over batch, 4 batches per core, weights replicated.

v2 design (per core: B=4 batches, SEQ=4096, H=512), all bf16 compute:
  - value loaded HBM->SBUF with f32->bf16 cast DMA (SWDGE) in 2-block pair
    tiles v_pair[128p, 2blk, 4j, 512h], s = BLK*512 + p*4 + j (8KB DRAM
    runs, 2.1MB reads -> good DMA efficiency), prefetched one batch ahead.
  - one xbar DMA-transpose (HWDGE) per 512-seq block:
    [128, 2048] -> vT[128p, 16jj, 128s2], jj = j*4 + k, h = 128k + p.
  - main mm k-outer for stationary reuse: for (g=4-blk group, hoc, k):
    LDW(Wv[k,hoc]) then 4 accumulating MMs (one per blk) -> psum[128,512].
  - tanh on ACT with per-partition bias qcombT (query projection is free).
  - score MMs col-tiled: 4 blocks of a group write rows 0/32/64/96 of ONE
    psum bank via tile_position=(0,32*blk) -> they run concurrently on PE.
  - exp directly from score psum on ACT: esc2d[8blk, 512] bf16 (no [1,4096]
    single-partition exp, no DVE scrow copies).
  - escT via 8 selector matmuls (lhsT=esc97[g][:,128j:...], rhs=sel[g]) ->
    psum[128, 4, 8] -> one DVE copy (replaces 32 tiny MMs + 32 copies).
  - total = ones128^T @ escT (1 tiny MM) -> DVE reduce -> reciprocal.
  - context: 32 accumulating MMs escT[:,j,row]^T @ v_pair slice -> [1,512];
    scale by 1/total on DVE; DMA out.
  - batch tail (escT/tot/ctx/store) deferred into the NEXT batch's main mm
    stream so PE never drains; scores for (g,hoc) deferred one hoc slot so
    they never wait on tanh.
  - weights/query/biases loaded via HWDGE (scalar queue, parallel with
    SWDGE value stream) as f32 + DVE cast; setup MMs run before the first
    main MM on an otherwise idle PE; no nested pool-exit barriers (they
    poisoned the sync queue in the previous version, stalling the first
    transpose to t=53us).
"""

import numpy as np

N_CORES = 8
BATCH_TOTAL = 32
B = BATCH_TOTAL // N_CORES  # batches per core
SEQ = 4096
H = 512
HC = H // 128   # 4 hidden chunks
NBLK = SEQ // 512   # 8 seq blocks per batch
NPAIR = NBLK // 2   # pair-granular value loads

_cache = {}


def build_nc(b_per_core=B, seq=SEQ):
    import concourse.bass as bass
    import concourse.mybir as mybir
    import concourse.tile as tile
    from concourse import bacc
    from concourse.masks import make_identity

    f32 = mybir.dt.float32
    bf16 = mybir.dt.bfloat16
    AF = mybir.ActivationFunctionType
    AX = mybir.AxisListType
    ALU = mybir.AluOpType

    nblk = seq // 512
    npair = nblk // 2

    nc = bacc.Bacc("TRN2", target_bir_lowering=False, debug=False)

    val_d = nc.dram_tensor("value", [b_per_core, seq, H], f32, kind="ExternalInput").ap()
    q_d = nc.dram_tensor("query", [b_per_core, H], f32, kind="ExternalInput").ap()
    Wq_d = nc.dram_tensor("Wq", [H, H], f32, kind="ExternalInput").ap()
    bq_d = nc.dram_tensor("bq", [H], f32, kind="ExternalInput").ap()
    Wv_d = nc.dram_tensor("Wv", [H, H], f32, kind="ExternalInput").ap()
    bv_d = nc.dram_tensor("bv", [H], f32, kind="ExternalInput").ap()
    Wo_d = nc.dram_tensor("Wo", [H, 1], f32, kind="ExternalInput").ap()
    bo_d = nc.dram_tensor("bo", [1], f32, kind="ExternalInput").ap()  # unused (cancels)
    out_d = nc.dram_tensor("out", [b_per_core, H], f32, kind="ExternalOutput").ap()

    # s = (g*4 + blk)*512 + p*4 + j  -> [b, g, p, blk, j, h] group tiles
    val_v = val_d.rearrange(
        "b (g blk p j) h -> b g p blk j h", g=2, blk=4, p=128, j=4
    )
    # chunked weight rows (match xbar layout h = 128k + p)
    Wv_v = Wv_d.rearrange("(k p) o -> p k o", p=128)
    Wq_v = Wq_d.rearrange("(k p) o -> p k o", p=128)
    Wo_nat_v = Wo_d.rearrange("(r c) one -> r (c one)", c=128)  # [4, 128]
    bq_v = bq_d.rearrange("(r c) -> r c", c=128)                # [4, 128]
    bv_v = bv_d.rearrange("(r c) -> r c", c=128)

    with tile.TileContext(nc) as tc:
        with (
            tc.tile_pool(name="weights", bufs=1) as wpool,
            tc.tile_pool(name="wf32", bufs=1) as wfpool,
            tc.tile_pool(name="vnat", bufs=5) as vpool,
            tc.tile_pool(name="vt", bufs=6) as tpool,
            tc.tile_pool(name="ht", bufs=3) as hpool,
            tc.tile_pool(name="small", bufs=6) as smpool,
            tc.tile_pool(name="psum_h", bufs=4, space="PSUM") as psh,
            tc.tile_pool(name="psum_sc", bufs=2, space="PSUM") as pss,
            tc.tile_pool(name="psum_ctx", bufs=1, space="PSUM") as psc,
            tc.tile_pool(name="psum_e", bufs=1, space="PSUM") as pse_pool,
        ):
            # ---- issue the value loads for batch 0 first (critical path) ----
            vgroups = {}  # (b, g) -> tile [128, 4blk, 4j, 512h] bf16
            def load_batch(b, split=False):
                for g in range(2):
                    vt = vpool.tile([128, 4, 4, H], bf16, tag="vnat", name="vg")
                    if split:
                        nc.gpsimd.dma_start(out=vt[:, 0:2], in_=val_v[b, g, :, 0:2])
                        nc.gpsimd.dma_start(out=vt[:, 2:4], in_=val_v[b, g, :, 2:4])
                    else:
                        nc.gpsimd.dma_start(out=vt[:], in_=val_v[b, g])
                    vgroups[(b, g)] = vt

            def load_one(b, g):
                vt = vpool.tile([128, 4, 4, H], bf16, tag="vnat", name="vg1")
                nc.gpsimd.dma_start(out=vt[:], in_=val_v[b, g])
                vgroups[(b, g)] = vt

            # ---- persistent SBUF residents ----
            Wv_sb = wpool.tile([128, HC, H], bf16)
            Wq_sb = wpool.tile([128, HC, H], bf16)
            Wo_sb = wpool.tile([128, HC], bf16)
            qcombT = wpool.tile([128, HC, b_per_core], f32)
            ones128 = wpool.tile([128, 1], bf16)
            id4 = wpool.tile([4, 4], bf16)
            id4f = wpool.tile([4, 4], f32)
            qT = wpool.tile([128, HC, b_per_core], bf16)
            bqvT = wpool.tile([128, HC], f32)
            q_nat = wpool.tile([b_per_core, H], bf16)
            wo_nat = wpool.tile([4, 128], bf16)
            bq_s = wpool.tile([4, 128], f32)
            bv_s = wpool.tile([4, 128], f32)
            bqv = wpool.tile([4, 128], f32)
            # exp outputs per group: rows 0/32/64/96 (same partitions as the
            # col-tiled score psum rows -> no cross-partition ACT moves);
            # sel_g picks those rows out in the escT transpose matmuls.
            esc97 = [wpool.tile([97, H], bf16, name=f"esc97_{g}") for g in range(2)]
            sel = [wpool.tile([97, 8], bf16, name=f"sel_{g}") for g in range(2)]
            warm = wpool.tile([128, H], bf16)
            ctx97 = wpool.tile([97, H], bf16)

            # PE warmup: ~24 matmuls on a zeroed scratch keep the HAM busy
            # window filled while the first value chunk loads, so the real
            # matmul stream starts at 2.4GHz instead of 1.2.
            nc.gpsimd.memset(warm[:], 0.0)
            nc.gpsimd.memset(ctx97[:], 0.0)
            nc.gpsimd.memset(ones128[:], 1.0)
            make_identity(nc, id4[:])
            make_identity(nc, id4f[:])
            ps_warm = psh.tile([128, H], f32, tag="ph", name="pswarm")
            for i in range(32):
                nc.tensor.matmul(ps_warm[:], warm[:, 0:128], warm[:],
                                 start=True, stop=True)

            # value stream first on the SWDGE queue (critical path); the
            # setup loads ride the parallel HWDGE (scalar) ring as f32 and
            # get DVE-cast -- the early PE stall this causes is harmless
            # (PE has slack at the start), and it keeps the SWDGE queue
            # pure value traffic, which paces the whole pipeline.
            # batch-0 first half-chunk, then Wv as a SWDGE cast load (1MB,
            # ready ~10us, before the first transpose), then the rest of b0
            vt00 = vpool.tile([128, 4, 4, H], bf16, tag="vnat", name="vg00")
            nc.gpsimd.dma_start(out=vt00[:, 0:2], in_=val_v[0, 0, :, 0:2])
            vgroups[(0, 0)] = vt00
            nc.gpsimd.dma_start(out=Wv_sb[:], in_=Wv_v)
            nc.gpsimd.dma_start(out=vt00[:, 2:4], in_=val_v[0, 0, :, 2:4])
            vt01 = vpool.tile([128, 4, 4, H], bf16, tag="vnat", name="vg01")
            nc.gpsimd.dma_start(out=vt01[:, 0:2], in_=val_v[0, 1, :, 0:2])
            nc.gpsimd.dma_start(out=vt01[:, 2:4], in_=val_v[0, 1, :, 2:4])
            vgroups[(0, 1)] = vt01

            # scalar ring: Wq FIRST (the inline qcomb matmuls are the only
            # PE-FIFO blocker), then the tiny tensors
            wq_f32 = wfpool.tile([128, HC, H], f32, tag="wf32", name="wqf")
            nc.scalar.dma_start(out=wq_f32[:], in_=Wq_v)
            nc.vector.tensor_copy(Wq_sb[:], wq_f32[:])

            q_f32 = wfpool.tile([b_per_core, H], f32, tag="qf32")
            nc.scalar.dma_start(out=q_f32[:], in_=q_d)
            nc.vector.tensor_copy(q_nat[:], q_f32[:])

            wo_f32 = wfpool.tile([4, 128], f32, tag="wof32")
            nc.scalar.dma_start(out=wo_f32[:], in_=Wo_nat_v)
            nc.vector.tensor_copy(wo_nat[:], wo_f32[:])

            nc.scalar.dma_start(out=bq_s[:], in_=bq_v)
            nc.scalar.dma_start(out=bv_s[:], in_=bv_v)

            # prefetch batch 1 now that setup loads are queued
            if b_per_core > 1:
                load_batch(1)
            for g in range(2):
                nc.gpsimd.memset(esc97[g][:], 0.0)
                nc.gpsimd.memset(sel[g][:], 0.0)
                for a in range(4):
                    nc.gpsimd.memset(sel[g][32 * a:32 * a + 1, g * 4 + a:g * 4 + a + 1], 1.0)

            # Wo^T: PE-transpose [4,128] -> [128, 4]
            ps_wo = psh.tile([128, HC], f32, tag="ph", name="pswo")
            nc.tensor.matmul(ps_wo[:], wo_nat[:], id4[:], start=True, stop=True)
            nc.vector.tensor_copy(Wo_sb[:], ps_wo[:])

            # (bq+bv)^T via PE transpose (fp32, tiny)
            nc.vector.tensor_add(bqv[:], bq_s[:], bv_s[:])
            ps_b = psh.tile([128, HC], f32, tag="ph", name="psb")
            nc.tensor.matmul(ps_b[:], bqv[:], id4f[:], start=True, stop=True)
            nc.vector.tensor_copy(bqvT[:], ps_b[:])

            # q^T chunks: [128, B] per hic
            for hic in range(HC):
                ps_q = psh.tile([128, b_per_core], f32, tag="ph", name="psq")
                nc.tensor.matmul(
                    ps_q[:], q_nat[0:b_per_core, 128 * hic:128 * (hic + 1)],
                    id4[0:b_per_core, 0:b_per_core], start=True, stop=True,
                )
                nc.vector.tensor_copy(qT[:, hic, :], ps_q[:])

            # qcombT[ho, b] = (q[b] @ Wq)[ho] + bq[ho] + bv[ho]
            if True:
                for hoc in range(HC):
                    ps_qp = psh.tile([128, b_per_core], f32, tag="ph", name="psqp")
                    for hic in range(HC):
                        nc.tensor.matmul(
                            ps_qp[:], Wq_sb[:, hic, 128 * hoc:128 * (hoc + 1)],
                            qT[:, hic, :], start=(hic == 0), stop=(hic == HC - 1),
                        )
                    nc.scalar.activation(
                        qcombT[:, hoc, :], ps_qp[:], AF.Identity,
                        bias=bqvT[:, hoc:hoc + 1],
                    )

            # ---------------- main pipeline ----------------
            # deferred-emit state: scores trail their tanh by TWO hoc slots
            # so they never catch a lagging ACT queue (esp. at the batch-0
            # boundary where ACT starts ~15us late)
            from collections import deque
            pending_q = deque()    # (p, hoc, hT_p, ps_sc)
            pending_exp = None     # (g, ps_sc_g)
            tails = {}             # b -> escT_sb tile

            def emit_scores_and_exp(flush=False):
                nonlocal pending_exp
                while pending_q and (flush or len(pending_q) > 0):
                    p_, hoc_, hT_p_, ps_sc_ = pending_q.popleft()
                    for bi in range(2):
                        row = 32 * ((2 * p_ + bi) % 4)
                        nc.tensor.matmul(
                            ps_sc_[row:row + 1, :],
                            Wo_sb[:, hoc_:hoc_ + 1],
                            hT_p_[:, hoc_, bi, :],
                            start=(hoc_ == 0), stop=(hoc_ == HC - 1),
                            tile_position=(0, row),
                        )
                    if hoc_ == HC - 1 and p_ % 2 == 1:
                        pending_exp = (p_ // 2, ps_sc_)
                if pending_exp is not None:
                    g_, ps_sc_ = pending_exp
                    for bb in range(4):
                        nc.scalar.activation(
                            esc97[g_][32 * bb:32 * bb + 1, :],
                            ps_sc_[32 * bb:32 * bb + 1, :], AF.Exp,
                        )
                    pending_exp = None

            def emit_tail_a(b_):
                """escT transposes + psum->sbuf copy for batch b_."""
                pse = pse_pool.tile([128, HC, 8], f32, tag="pse")
                for j in range(HC):
                    for g_ in range(2):
                        nc.tensor.matmul(
                            pse[:, j, :], esc97[g_][:, 128 * j:128 * (j + 1)],
                            sel[g_][:], start=(g_ == 0), stop=(g_ == 1),
                        )
                escT_sb = smpool.tile([128, HC, 8], bf16, tag="escT")
                nc.vector.tensor_copy(escT_sb[:], pse[:])
                tails[b_] = escT_sb

            def emit_tail_b(b_):
                """total + reciprocal + context + store for batch b_."""
                escT_sb = tails.pop(b_)
                tot_ps = pse_pool.tile([1, HC * 8], f32, tag="pse", name="totps")
                nc.tensor.matmul(
                    tot_ps[:], ones128[:],
                    escT_sb[:].rearrange("p j r -> p (j r)"),
                    start=True, stop=True,
                )
                tot1 = smpool.tile([1, 1], f32, tag="tot1")
                nc.vector.tensor_reduce(tot1[:], tot_ps[:], axis=AX.X, op=ALU.add)
                rec = smpool.tile([1, 1], f32, tag="rec")
                nc.vector.reciprocal(rec[:], tot1[:])
                ps_ctx = psc.tile([128, H], f32, tag="ctx")
                for blk in range(nblk):
                    for j in range(HC):
                        nc.tensor.matmul(
                            ps_ctx[32 * j:32 * j + 1, :],
                            escT_sb[:, j, blk:blk + 1],
                            vgroups[(b_, blk // 4)][:, blk % 4, j, :],
                            start=(blk == 0), stop=(blk == nblk - 1),
                            tile_position=(0, 32 * j),
                        )
                for j in range(HC):
                    nc.vector.tensor_copy(
                        ctx97[32 * j:32 * j + 1, :], ps_ctx[32 * j:32 * j + 1, :],
                    )
                ps_cs = pse_pool.tile([1, H], f32, tag="pse", name="pscs")
                nc.tensor.matmul(
                    ps_cs[:], ones128[0:97, :], ctx97[:], start=True, stop=True,
                )
                outrow = smpool.tile([1, H], f32, tag="outrow")
                nc.vector.tensor_scalar_mul(outrow[:], ps_cs[:], rec[:])
                nc.gpsimd.dma_start(out=out_d[b_:b_ + 1, :], in_=outrow[:])
                # release value tiles of b_
                for g_ in range(2):
                    del vgroups[(b_, g_)]

            # one xbar transpose per 2-block pair (1MB ops, short vT
            # lifetime); emitted one batch AHEAD (mid-previous-batch) so the
            # sync ring pre-transposes during the previous batch's tail and
            # the next batch's main matmuls never wait on the xbar
            vTss = {}
            def emit_transposes(b):
                lst = []
                for p in range(4):
                    vT = tpool.tile([128, 8 * HC, 128], bf16, tag="vt")
                    nc.sync.dma_start_transpose(
                        out=vT[:],
                        in_=vgroups[(b, p // 2)][:, (p % 2) * 2:(p % 2) * 2 + 2],
                    )
                    lst.append(vT)
                vTss[b] = lst

            emit_transposes(0)
            if b_per_core > 1:
                emit_transposes(1)

            for b in range(b_per_core):
                vTps = vTss.pop(b)

                ps_sc_g = None
                for p in range(4):
                    hT_p = hpool.tile([128, HC, 2, H], bf16, tag="ht")
                    if p % 2 == 0:
                        ps_sc_g = pss.tile([128, H], f32, tag="sc")
                    # chunk c = bi*16 + j*4 + k in the pair transpose
                    vTv = vTps[p][:].rearrange(
                        "p (bi j k) s -> p bi k j s", bi=2, k=HC
                    )
                    for hoc in range(HC):
                        # main MMs: k-outer, stationary Wv[k,hoc] reused
                        # across the 2 blocks of the pair
                        ps2 = [
                            psh.tile([128, H], f32, tag="ph", name=f"ps2_{i}")
                            for i in range(2)
                        ]
                        for k in range(HC):
                            for bi in range(2):
                                nc.tensor.matmul(
                                    ps2[bi][:],
                                    Wv_sb[:, k, 128 * hoc:128 * (hoc + 1)],
                                    vTv[:, bi, k],
                                    start=(k == 0), stop=(k == HC - 1),
                                )
                        # deferred score MMs (previous slot) + exp
                        emit_scores_and_exp()
                        # tanh with query-projection bias
                        for bi in range(2):
                            nc.scalar.activation(
                                hT_p[:, hoc, bi, :], ps2[bi][:], AF.Tanh,
                                bias=qcombT[:, hoc, b:b + 1],
                            )
                        pending_q.append((p, hoc, hT_p, ps_sc_g))
                        # interleave previous batch's tail into this stream
                        if p == 0 and hoc == 1 and b > 0:
                            emit_tail_a(b - 1)
                        if p == 0 and hoc == 2 and (b - 1) in tails:
                            emit_tail_b(b - 1)
                        if p in (1, 2) and hoc == 0 and b + 2 < b_per_core:
                            load_one(b + 2, p - 1)
                        if p == 2 and hoc == 1 and b + 2 <= b_per_core - 1:
                            emit_transposes(b + 2)

            # drain: last group's scores + exp, then last batch tail
            emit_scores_and_exp(flush=True)
            last = b_per_core - 1
            emit_tail_a(last)
            emit_tail_b(last)

    nc.compile()
    return nc


def kernel(**inputs):
    from concourse.bass_utils import run_bass_kernel_spmd

    key = "full"
    if key not in _cache:
        _cache[key] = build_nc()
    nc = _cache[key]

    query = np.asarray(inputs["query"], dtype=np.float32)   # [1, 32, 512]
    value = np.asarray(inputs["value"], dtype=np.float32)   # [32, 4096, 512]
    Wq = np.asarray(inputs["Wq"], dtype=np.float32)
    bq = np.asarray(inputs["bq"], dtype=np.float32)
    Wv = np.asarray(inputs["Wv"], dtype=np.float32)
    bv = np.asarray(inputs["bv"], dtype=np.float32)
    Wo = np.asarray(inputs["Wo"], dtype=np.float32)
    bo = np.asarray(inputs["bo"], dtype=np.float32)

    in_maps = []
    for i in range(N_CORES):
        sl = slice(B * i, B * (i + 1))
        in_maps.append({
            "value": np.ascontiguousarray(value[sl]),
            "query": np.ascontiguousarray(query[0, sl, :]),
            "Wq": Wq, "bq": bq, "Wv": Wv, "bv": bv, "Wo": Wo, "bo": bo,
        })

    res = run_bass_kernel_spmd(nc, in_maps, core_ids=list(range(N_CORES)))
    out = np.concatenate([res.results[i]["out"] for i in range(N_CORES)], axis=0)
    return out[:, None, :].astype(np.float32)  # [32, 1, 512]



# revision 5
# speedup vs baseline: 1.1210x; 1.1210x over previous
"""AdditiveAttention (Bahdanau) distributed Bass kernel for 8 TRN2 NeuronCores.

Computation (per batch b):
    qc[b,:]   = query[b] @ Wq + bq + bv                       # [512]  (HOST)
    z[b,s,:]  = value[b,s] @ Wv + qc[b]                       # pre-tanh
    score     = tanh(z) @ Wo          (+bo dropped: cancels in softmax)
    align     = softmax(score)        (no max-sub: |score| <= ~23)
    out[b,:]  = align @ value[b]

Sharding: data-parallel over batch, 4 batches per core, weights replicated.

v3 design (per core: B=4 batches, SEQ=4096, H=512):
  - HOST prep: qcombT (q-projection, transposed), Wo strip-padded to 32
    cols, selector matrices, and Wv pre-scaled x32 + packed to fp8e4 in
    the DoubleRow interleave h = 256*g + 2*p + i.
  - value: SWDGE cast-DMA f32->bf16 natural group tiles (context path),
    then DVE cast bf16->fp8 natural, then ONE xbar DMA-transpose per
    2-block pair of the fp8 data VIEWED AS bf16 pairs -> vT8 where each
    16-bit unit holds (h, h+1) adjacent fp8 values: exactly DoubleRow's
    moving-operand format. Transposed DMA bytes halve vs bf16 (8.4 MB).
  - main z matmuls: fp8 DoubleRow, K=256/pass -> 4 MMs of N=512 per
    (pair, hoc) accumulating a [128,1024] 2-bank PSUM tile (64 MMs/batch
    vs 128 bf16 MMs in v2).
  - tanh on ACT, N=1024 per instruction (vs 512), scale=1/32 undoes the
    Wv prescale, bias=qcombT adds the query projection.
  - scores: RAW (pre-exp) score strips accumulated in one PSUM bank per
    4-block group, 4-way col-tiled (M=32 strips so all 97 partitions are
    written), then ONE DVE copy -> s97, selector MMs transpose raw
    scores, and ONE exp per batch [128,32] with accum_out giving the
    softmax total for free. (v2 spent 23us/core on [1,512] exp calls.)
  - context: 32 accumulating MMs escT^T @ v_nat -> [1,512] rows, scale
    by 1/total, store via sync-queue DMA.
  - pipeline: slot calendar (b,pair,hoc); scores trail tanh by 2 slots;
    tails and next-batch value pipeline interleave into the z stream.
"""

import numpy as np

N_CORES = 8
BATCH_TOTAL = 32
B = BATCH_TOTAL // N_CORES  # batches per core
SEQ = 4096
H = 512
HC = H // 128   # 4 hidden chunks
NBLK = SEQ // 512   # 8 seq blocks per batch
NPAIR = NBLK // 2
WV_SCALE = 32.0

_cache = {}


def build_nc(b_per_core=B, seq=SEQ):
    from collections import deque

    import concourse.bass as bass
    import concourse.mybir as mybir
    import concourse.tile as tile
    from concourse import bacc

    f32 = mybir.dt.float32
    bf16 = mybir.dt.bfloat16
    fp8 = mybir.dt.float8e4
    AF = mybir.ActivationFunctionType
    DR = mybir.MatmulPerfMode.DoubleRow

    nblk = seq // 512
    npair = nblk // 2

    nc = bacc.Bacc("TRN2", target_bir_lowering=False, debug=False)

    val_d = nc.dram_tensor("value", [b_per_core, seq, H], f32, kind="ExternalInput").ap()
    wv8_d = nc.dram_tensor("wv8", [128, 2, 2, H], fp8, kind="ExternalInput").ap()
    qct_d = nc.dram_tensor("qcombT", [128, HC, b_per_core], f32, kind="ExternalInput").ap()
    wo32_d = nc.dram_tensor("wo32", [128, HC, 32], bf16, kind="ExternalInput").ap()
    sel_d = nc.dram_tensor("sel", [2, 97, 8], f32, kind="ExternalInput").ap()
    out_d = nc.dram_tensor("out", [b_per_core, H], f32, kind="ExternalOutput").ap()

    # s = (g2*4 + blk)*512 + p*4 + j
    val_v = val_d.rearrange(
        "b (g blk p j) h -> b g p blk j h", g=2, blk=4, p=128, j=4
    )

    with tile.TileContext(nc) as tc:
        with (
            tc.tile_pool(name="weights", bufs=1) as wpool,
            tc.tile_pool(name="vnat", bufs=6) as vpool,
            tc.tile_pool(name="v8", bufs=3) as v8pool,
            tc.tile_pool(name="vt8", bufs=7) as tpool,
            tc.tile_pool(name="ht", bufs=9) as hpool,
            tc.tile_pool(name="small", bufs=6) as smpool,
            tc.tile_pool(name="psum_z", bufs=3, space="PSUM") as psz,
            tc.tile_pool(name="psum_sc", bufs=1, space="PSUM") as pssc,
            tc.tile_pool(name="psum_tl", bufs=1, space="PSUM") as pstl,
        ):
            # ---- persistent SBUF residents ----
            Wv8_sb = wpool.tile([128, 2, 2, H], fp8)
            qcT = wpool.tile([128, HC, b_per_core], f32)
            Wo32_sb = wpool.tile([128, HC, 32], bf16)
            sel_sb = wpool.tile([97, 2, 8], f32)
            ones128 = wpool.tile([128, 1], bf16)
            ones128f = wpool.tile([128, 1], f32)
            s97 = [wpool.tile([97, H], f32, name=f"s97_{g}") for g in range(2)]
            ctx97 = wpool.tile([97, H], bf16)
            warm = wpool.tile([128, H], bf16)
            prew_out = wpool.tile([1, 32], f32)

            # ---- value pipeline ----
            vnats = {}   # (b, g2) -> bf16 natural tile [128, 4, 4, 512]
            v8s = {}     # (b, g2) -> fp8 natural tile  [128, 4, 2, 4, 256]
            vT8s = {}    # (b, p)  -> packed transpose  [128, 16, 128] bf16 units

            def load_group(b, g2):
                vt = vpool.tile([128, 4, 4, H], bf16, tag="vnat", name="vg")
                nc.gpsimd.dma_start(out=vt[:, 0:2], in_=val_v[b, g2, :, 0:2])
                nc.gpsimd.dma_start(out=vt[:, 2:4], in_=val_v[b, g2, :, 2:4])
                vnats[(b, g2)] = vt

            def cast_pair(b, pair):
                g2, q = pair // 2, pair % 2
                if (b, g2) not in v8s:
                    v8s[(b, g2)] = v8pool.tile(
                        [128, 4, 2, 4, 256], fp8, tag="v8", name="v8g")
                v8 = v8s[(b, g2)]
                vt = vnats[(b, g2)]
                for g in range(2):
                    nc.vector.tensor_copy(
                        v8[:, 2 * q:2 * q + 2, g],
                        vt[:, 2 * q:2 * q + 2, :, 256 * g:256 * (g + 1)])

            def transpose_pair(b, p):
                vT = tpool.tile([128, 16, 128], bf16, tag="vt8", name="vt8")
                src = v8s[(b, p // 2)][:].bitcast(bf16)[:, (p % 2) * 2:(p % 2) * 2 + 2]
                nc.sync.dma_start_transpose(out=vT[:], in_=src)
                vT8s[(b, p)] = vT

            # prologue — value-load triggers FIRST on the SWDGE queue (the
            # critical path), memsets/weights/warmup behind them; batch-0
            # casts+transposes pair-pipelined so the first z MM starts as
            # soon as 1 MB has landed, not after the full 2 MB group.
            load_group(0, 0)
            nc.gpsimd.memset(warm[:], 0.0)
            cast_pair(0, 0)
            transpose_pair(0, 0)
            # ACT table prewarm: exp+tanh live in one set; load it while
            # the first value tiles stream in, not at first real tanh.
            nc.scalar.activation(prew_out[:], warm[0:1, 0:32], AF.Exp)
            nc.scalar.activation(prew_out[:], warm[0:1, 0:32], AF.Tanh)
            # weights ride the scalar (ACT) HWDGE queue: off the SWDGE
            # value-load critical path, done within a few us.
            nc.scalar.dma_start(out=Wv8_sb[:], in_=wv8_d)
            nc.scalar.dma_start(out=qcT[:], in_=qct_d)
            nc.scalar.dma_start(out=Wo32_sb[:], in_=wo32_d)
            nc.scalar.dma_start(out=sel_sb[:], in_=sel_d.rearrange("g p r -> p g r"))
            cast_pair(0, 1)
            transpose_pair(0, 1)
            load_group(0, 1)
            cast_pair(0, 2)
            transpose_pair(0, 2)
            cast_pair(0, 3)
            transpose_pair(0, 3)
            if b_per_core > 1:
                load_group(1, 0)
                load_group(1, 1)
            nc.gpsimd.memset(ones128[:], 1.0)
            nc.gpsimd.memset(ones128f[:], 1.0)
            nc.gpsimd.memset(ctx97[:], 0.0)

            # PE warmup: fill the HAM activity window while value loads run.
            ps_warm = pstl.tile([128, H], f32, tag="tl", name="pswarm")
            for _ in range(24):
                nc.tensor.matmul(ps_warm[:], warm[:, 0:128], warm[:],
                                 start=True, stop=True)

            # ---- deferred-emission machinery ----
            pending_scores = deque()  # (b, g2, hoc, ready_slot)
            pending_tail = deque()    # (kind, b, g2, ready_slot)
            hTs = {}
            score_banks = {}
            tails = {}

            def emit_scores(t):
                while pending_scores and pending_scores[0][3] <= t:
                    b_, g2, hoc, _ = pending_scores.popleft()
                    if hoc == 0:
                        score_banks[(b_, g2)] = pssc.tile([128, H], f32, tag="sc", name="ssum")
                    ssum = score_banks[(b_, g2)]
                    for pp in range(2):
                        hT = hTs.pop((b_, 2 * g2 + pp, hoc))
                        for bi in range(2):
                            row = 32 * (2 * pp + bi)
                            nc.tensor.matmul(
                                ssum[row:row + 32, :], Wo32_sb[:, hoc, :],
                                hT[:, 512 * bi:512 * (bi + 1)],
                                start=(hoc == 0), stop=(hoc == HC - 1),
                                tile_position=(0, row),
                            )
                    if hoc == HC - 1:
                        pending_tail.append(("s97", b_, g2, t + 1))

            def emit_tail(t):
                if not (pending_tail and pending_tail[0][3] <= t):
                    return
                kind, b_, g2, _ = pending_tail.popleft()
                if kind == "s97":
                    ssum = score_banks.pop((b_, g2))
                    nc.vector.tensor_copy(s97[g2][:], ssum[0:97, :])
                    if g2 == 1:
                        pending_tail.append(("taila", b_, None, t + 1))
                elif kind == "taila":
                    pse = pstl.tile([128, HC, 8], f32, tag="tl", name="pse")
                    for j in range(HC):
                        for g2_ in range(2):
                            nc.tensor.matmul(
                                pse[:, j, :], s97[g2_][:, 128 * j:128 * (j + 1)],
                                sel_sb[:, g2_, :], start=(g2_ == 0), stop=(g2_ == 1),
                            )
                    escT = smpool.tile([128, HC, 8], bf16, tag="escT", name="escT")
                    eacc = smpool.tile([128, 1], f32, tag="eacc", name="eacc")
                    nc.scalar.activation(escT[:], pse[:], AF.Exp, accum_out=eacc[:])
                    tails[b_] = (escT, eacc)
                    pending_tail.append(("tailb", b_, None, t + 1))
                elif kind == "tailb":
                    escT, eacc = tails.pop(b_)
                    tot_ps = pstl.tile([1, 1], f32, tag="tl", name="totps")
                    nc.tensor.matmul(tot_ps[:], eacc[:], ones128f[:],
                                     start=True, stop=True)
                    rec = smpool.tile([1, 1], f32, tag="rec", name="rec")
                    nc.vector.reciprocal(rec[:], tot_ps[:])
                    ctx_ps = pstl.tile([128, H], f32, tag="tl", name="ctxps")
                    for blk in range(nblk):
                        for j in range(HC):
                            nc.tensor.matmul(
                                ctx_ps[32 * j:32 * j + 1, :],
                                escT[:, j, blk:blk + 1],
                                vnats[(b_, blk // 4)][:, blk % 4, j, :],
                                start=(blk == 0), stop=(blk == nblk - 1),
                                tile_position=(0, 32 * j),
                            )
                    for j in range(HC):
                        nc.vector.tensor_copy(
                            ctx97[32 * j:32 * j + 1, :], ctx_ps[32 * j:32 * j + 1, :],
                        )
                    cs_ps = pstl.tile([1, H], f32, tag="tl", name="csps")
                    nc.tensor.matmul(cs_ps[:], ones128[0:97, :], ctx97[:],
                                     start=True, stop=True)
                    outrow = smpool.tile([1, H], f32, tag="outrow", name="outrow")
                    nc.vector.tensor_scalar_mul(outrow[:], cs_ps[:], rec[:])
                    nc.sync.dma_start(out=out_d[b_:b_ + 1, :], in_=outrow[:])
                    del vnats[(b_, 0)], vnats[(b_, 1)]

            # value pipeline calendar during batch b: casts+transposes for
            # b+1 (loads landed a batch ago, so DVE never blocks), loads
            # for b+2 (a full 1.5 batches ahead of their casts).
            def cal_events(b, p, hoc):
                nb, nnb = b + 1, b + 2
                k = (p, hoc)
                if nb < b_per_core:
                    if k == (0, 1):
                        cast_pair(nb, 0)
                    elif k == (0, 2):
                        transpose_pair(nb, 0)
                    elif k == (0, 3):
                        cast_pair(nb, 1)
                    elif k == (1, 1):
                        transpose_pair(nb, 1)
                    elif k == (1, 2):
                        cast_pair(nb, 2)
                    elif k == (1, 3):
                        transpose_pair(nb, 2)
                    elif k == (2, 1):
                        cast_pair(nb, 3)
                    elif k == (2, 2):
                        transpose_pair(nb, 3)
                if nnb < b_per_core:
                    if k == (2, 0):
                        load_group(nnb, 0)
                    elif k == (3, 0):
                        load_group(nnb, 1)

            # ---------------- main pipeline ----------------
            t = 0
            for b in range(b_per_core):
                for p in range(npair):
                    vt8 = vT8s.pop((b, p))
                    vt8f = vt8[:].bitcast(fp8)   # [128, 16, 256]
                    for hoc in range(HC):
                        emit_scores(t)
                        emit_tail(t)
                        cal_events(b, p, hoc)
                        zp = psz.tile([128, 1024], f32, tag="z", name="zp")
                        for g in range(2):
                            lhsT = Wv8_sb[:, g, :, 128 * hoc:128 * (hoc + 1)]
                            for bi in range(2):
                                c0 = 4 * (2 * bi + g)
                                rhs = vt8f[:, c0:c0 + 4].rearrange(
                                    "p j (s i) -> p i (j s)", i=2
                                )
                                nc.tensor.matmul(
                                    zp[:, 512 * bi:512 * (bi + 1)], lhsT, rhs,
                                    start=(g == 0), stop=(g == 1), perf_mode=DR,
                                )
                        hT = hpool.tile([128, 1024], bf16, tag="ht", name="hT")
                        nc.scalar.activation(
                            hT[:], zp[:], AF.Tanh,
                            bias=qcT[:, hoc, b:b + 1], scale=1.0 / WV_SCALE,
                        )
                        hTs[(b, p, hoc)] = hT
                        if p % 2 == 1:
                            pending_scores.append((b, p // 2, hoc, t + 2))
                        t += 1

            # drain
            while pending_scores or pending_tail:
                emit_scores(t)
                emit_tail(t)
                t += 1

    nc.compile()
    return nc


def make_in_maps(query, value, Wq, bq, Wv, bv, Wo, bo):
    """Host-side prep: shard + precompute small tensors. query [1,32,512]."""
    import ml_dtypes

    query = np.asarray(query, dtype=np.float32)
    value = np.asarray(value, dtype=np.float32)
    Wq = np.asarray(Wq, dtype=np.float32)
    bq = np.asarray(bq, dtype=np.float32)
    Wv = np.asarray(Wv, dtype=np.float32)
    bv = np.asarray(bv, dtype=np.float32)
    Wo = np.asarray(Wo, dtype=np.float32)

    # qcomb[b, h] = q[b] @ Wq + bq + bv
    qcomb = query[0] @ Wq + bq + bv                    # [32, 512]
    # Wv8[p, g, i, ho] = fp8(32 * Wv[256g + 2p + i, ho])
    wv_s = (WV_SCALE * Wv).reshape(2, 128, 2, H)       # [g, p, i, ho]
    wv8 = np.ascontiguousarray(
        wv_s.transpose(1, 0, 2, 3)).astype(ml_dtypes.float8_e4m3)  # [128,2,2,H]
    # Wo strips: [128, hc, 32], col 0 = Wo chunk, rest 0
    wo32 = np.zeros((128, HC, 32), np.float32)
    wo32[:, :, 0] = Wo[:, 0].reshape(HC, 128).T
    wo32 = wo32.astype(ml_dtypes.bfloat16)
    # selectors [2, 97, 8]
    sel = np.zeros((2, 97, 8), np.float32)
    for g2 in range(2):
        for a in range(4):
            sel[g2, 32 * a, g2 * 4 + a] = 1.0

    in_maps = []
    for i in range(N_CORES):
        sl = slice(B * i, B * (i + 1))
        qcT = np.ascontiguousarray(
            qcomb[sl].reshape(B, HC, 128).transpose(2, 1, 0))  # [128, HC, B]
        in_maps.append({
            "value": np.ascontiguousarray(value[sl]),
            "wv8": wv8,
            "qcombT": qcT,
            "wo32": wo32,
            "sel": sel,
        })
    return in_maps


def kernel(**inputs):
    from concourse.bass_utils import run_bass_kernel_spmd

    key = "full"
    if key not in _cache:
        _cache[key] = build_nc()
    nc = _cache[key]

    in_maps = make_in_maps(
        inputs["query"], inputs["value"], inputs["Wq"], inputs["bq"],
        inputs["Wv"], inputs["bv"], inputs["Wo"], inputs["bo"],
    )
    res = run_bass_kernel_spmd(nc, in_maps, core_ids=list(range(N_CORES)))
    out = np.concatenate([res.results[i]["out"] for i in range(N_CORES)], axis=0)
    return out[:, None, :].astype(np.float32)  # [32, 1, 512]


# revision 12
# speedup vs baseline: 1.3522x; 1.2062x over previous
"""AdditiveAttention (Bahdanau) distributed Bass kernel for 8 TRN2 NeuronCores.

Computation (per batch b):
    qc[b,:]   = query[b] @ Wq + bq + bv                       # [512]  (HOST)
    z[b,s,:]  = value[b,s] @ Wv + qc[b]                       # pre-tanh
    score     = tanh(z) @ Wo          (+bo dropped: cancels in softmax)
    align     = softmax(score)        (no max-sub: |score| <= ~23)
    out[b,:]  = align @ value[b]

Sharding: data-parallel over batch, 4 batches per core, weights replicated.

v3 design (per core: B=4 batches, SEQ=4096, H=512):
  - HOST prep: qcombT (q-projection, transposed), Wo strip-padded to 32
    cols, selector matrices, and Wv pre-scaled x32 + packed to fp8e4 in
    the DoubleRow interleave h = 256*g + 2*p + i.
  - value: SWDGE cast-DMA f32->bf16 natural group tiles (context path),
    then DVE cast bf16->fp8 natural, then ONE xbar DMA-transpose per
    2-block pair of the fp8 data VIEWED AS bf16 pairs -> vT8 where each
    16-bit unit holds (h, h+1) adjacent fp8 values: exactly DoubleRow's
    moving-operand format. Transposed DMA bytes halve vs bf16 (8.4 MB).
  - main z matmuls: fp8 DoubleRow, K=256/pass -> 4 MMs of N=512 per
    (pair, hoc) accumulating a [128,1024] 2-bank PSUM tile (64 MMs/batch
    vs 128 bf16 MMs in v2).
  - tanh on ACT, N=1024 per instruction (vs 512), scale=1/32 undoes the
    Wv prescale, bias=qcombT adds the query projection.
  - scores: RAW (pre-exp) score strips accumulated in one PSUM bank per
    4-block group, 4-way col-tiled (M=32 strips so all 97 partitions are
    written), then ONE DVE copy -> s97, selector MMs transpose raw
    scores, and ONE exp per batch [128,32] with accum_out giving the
    softmax total for free. (v2 spent 23us/core on [1,512] exp calls.)
  - context: 32 accumulating MMs escT^T @ v_nat -> [1,512] rows, scale
    by 1/total, store via sync-queue DMA.
  - pipeline: slot calendar (b,pair,hoc); scores trail tanh by 2 slots;
    tails and next-batch value pipeline interleave into the z stream.
"""

import numpy as np

N_CORES = 8
BATCH_TOTAL = 32
B = BATCH_TOTAL // N_CORES  # batches per core
SEQ = 4096
H = 512
HC = H // 128   # 4 hidden chunks
NBLK = SEQ // 512   # 8 seq blocks per batch
NPAIR = NBLK // 2
WV_SCALE = 32.0

_cache = {}


def build_nc(b_per_core=B, seq=SEQ):
    from collections import deque

    import concourse.bass as bass
    import concourse.mybir as mybir
    import concourse.tile as tile
    from concourse import bacc
    from concourse.masks import make_identity

    f32 = mybir.dt.float32
    bf16 = mybir.dt.bfloat16
    fp8 = mybir.dt.float8e4
    AF = mybir.ActivationFunctionType
    DR = mybir.MatmulPerfMode.DoubleRow

    nblk = seq // 512
    npair = nblk // 2

    nc = bacc.Bacc("TRN2", target_bir_lowering=False, debug=False)

    val_d = nc.dram_tensor("value", [b_per_core, seq, H], f32, kind="ExternalInput").ap()
    wv8_d = nc.dram_tensor("wv8", [128, 2, 2, H], fp8, kind="ExternalInput").ap()
    qct_d = nc.dram_tensor("qcombT", [128, HC, b_per_core], f32, kind="ExternalInput").ap()
    wo32_d = nc.dram_tensor("wo32", [128, HC, 32], bf16, kind="ExternalInput").ap()
    sel_d = nc.dram_tensor("sel", [2, 97, 8], f32, kind="ExternalInput").ap()
    out_d = nc.dram_tensor("out", [b_per_core, H], f32, kind="ExternalOutput").ap()

    # s = (g2*4 + blk)*512 + p*4 + j
    val_v = val_d.rearrange(
        "b (g blk p j) h -> b g p blk j h", g=2, blk=4, p=128, j=4
    )

    with tile.TileContext(nc) as tc:
        with (
            tc.tile_pool(name="weights", bufs=1) as wpool,
            tc.tile_pool(name="vnat", bufs=6) as vpool,
            tc.tile_pool(name="v8", bufs=3) as v8pool,
            tc.tile_pool(name="vt8", bufs=7) as tpool,
            tc.tile_pool(name="ht", bufs=9) as hpool,
            tc.tile_pool(name="small", bufs=6) as smpool,
            tc.tile_pool(name="psum_z", bufs=2, space="PSUM") as psz,
            tc.tile_pool(name="psum_tr", bufs=2, space="PSUM") as ptr,
            tc.tile_pool(name="psum_sc", bufs=1, space="PSUM") as pssc,
            tc.tile_pool(name="psum_tl", bufs=1, space="PSUM") as pstl,
        ):
            # ---- persistent SBUF residents ----
            Wv8_sb = wpool.tile([128, 2, 2, H], fp8)
            qcT = wpool.tile([128, HC, b_per_core], f32)
            Wo32_sb = wpool.tile([128, HC, 32], bf16)
            sel_sb = wpool.tile([97, 2, 8], f32)
            ones128 = wpool.tile([128, 1], bf16)
            ones128f = wpool.tile([128, 1], f32)
            s97 = [wpool.tile([97, H], f32, name=f"s97_{g}") for g in range(2)]
            ctx97 = wpool.tile([97, H], bf16)
            warm = wpool.tile([128, H], bf16)
            prew_out = wpool.tile([1, 32], f32)
            ident = wpool.tile([128, 128], bf16)

            # ---- value pipeline ----
            vnats = {}   # (b, g2) -> bf16 natural tile [128, 4, 4, 512]
            v8s = {}     # (b, g2) -> fp8 natural tile  [128, 4, 2, 4, 256]
            vT8s = {}    # (b, p)  -> packed transpose  [128, 16, 128] bf16 units

            def load_group(b, g2):
                vt = vpool.tile([128, 4, 4, H], bf16, tag="vnat", name="vg")
                nc.gpsimd.dma_start(out=vt[:, 0:2], in_=val_v[b, g2, :, 0:2])
                nc.gpsimd.dma_start(out=vt[:, 2:4], in_=val_v[b, g2, :, 2:4])
                vnats[(b, g2)] = vt

            def cast_pair(b, pair):
                g2, q = pair // 2, pair % 2
                if (b, g2) not in v8s:
                    v8s[(b, g2)] = v8pool.tile(
                        [128, 4, 2, 4, 256], fp8, tag="v8", name="v8g")
                v8 = v8s[(b, g2)]
                vt = vnats[(b, g2)]
                for g in range(2):
                    nc.vector.tensor_copy(
                        v8[:, 2 * q:2 * q + 2, g],
                        vt[:, 2 * q:2 * q + 2, :, 256 * g:256 * (g + 1)])

            # PE-side transpose of the packed units: each [128,128] chunk of
            # the fp8-pair data (viewed as bf16 units) goes through matmul
            # transpose-mode into a bf16 PSUM bank (8 chunks per bank), then
            # one DVE copy lands it in vT8. No DMA-transposes at all: the
            # Tile framework serializes those against every other DMA (HW
            # deadlock guard), which lock-stepped loads and transposes into
            # a ~12us alternation in the v3a trace.
            def transpose_chunk(b, p, c, ps_tr):
                if (b, p) not in vT8s:
                    vT8s[(b, p)] = tpool.tile([128, 16, 128], bf16, tag="vt8",
                                              name="vt8")
                g2, q = p // 2, p % 2
                src = v8s[(b, g2)][:].bitcast(bf16).rearrange(
                    "p a b c d -> p (a b c d)")
                lo = q * 2048 + c * 128
                nc.tensor.matmul(ps_tr[:, c % 8, :], src[:, lo:lo + 128],
                                 ident[:], start=True, stop=True,
                                 is_transpose=True)

            def transpose_flush(b, p, c8, ps_tr):
                # copy chunks [c8, c8+8) of pair p from psum to vT8
                nc.vector.tensor_copy(vT8s[(b, p)][:, c8:c8 + 8, :], ps_tr[:])

            # transpose work queue: ("t", b, p, c) chunk transposes and
            # ("f", b, p, c8) psum->vT8 flushes, drained a few per slot.
            trans_q = deque()
            cur_ps = [None]

            def enqueue_transposes(b, p):
                for c in range(16):
                    trans_q.append(("t", b, p, c))
                    if c % 8 == 7:
                        trans_q.append(("f", b, p, c - 7))

            def drain_transposes(n):
                for _ in range(n):
                    if not trans_q:
                        return
                    it = trans_q.popleft()
                    if it[0] == "t":
                        _, b_, p_, c_ = it
                        if c_ % 8 == 0:
                            cur_ps[0] = ptr.tile([128, 8, 128], bf16,
                                                 tag="tr", name="pstr")
                        transpose_chunk(b_, p_, c_, cur_ps[0])
                    else:
                        _, b_, p_, c8 = it
                        transpose_flush(b_, p_, c8, cur_ps[0])

            # prologue — value-load triggers FIRST on the SWDGE queue (the
            # critical path; pure HBM loads now, so they free-run).
            load_group(0, 0)
            nc.gpsimd.memset(warm[:], 0.0)
            # ACT table prewarm: exp+tanh live in one set; load it while
            # the first value tiles stream in, not at first real tanh.
            nc.scalar.activation(prew_out[:], warm[0:1, 0:32], AF.Exp)
            nc.scalar.activation(prew_out[:], warm[0:1, 0:32], AF.Tanh)
            # weights ride the scalar (ACT) HWDGE queue: off the SWDGE
            # value-load critical path, done within a few us.
            nc.scalar.dma_start(out=Wv8_sb[:], in_=wv8_d)
            nc.scalar.dma_start(out=qcT[:], in_=qct_d)
            nc.scalar.dma_start(out=Wo32_sb[:], in_=wo32_d)
            nc.scalar.dma_start(out=sel_sb[:], in_=sel_d.rearrange("g p r -> p g r"))
            load_group(0, 1)
            if b_per_core > 1:
                load_group(1, 0)
                load_group(1, 1)
            nc.gpsimd.memset(ones128[:], 1.0)
            nc.gpsimd.memset(ones128f[:], 1.0)
            nc.gpsimd.memset(ctx97[:], 0.0)
            make_identity(nc, ident[:])

            # PE warmup: fill the HAM activity window while value loads run.
            ps_warm = pstl.tile([128, H], f32, tag="tl", name="pswarm")
            for _ in range(20):
                nc.tensor.matmul(ps_warm[:], warm[:, 0:128], warm[:],
                                 start=True, stop=True)

            # batch-0 casts + transposes up front; the PE transposes wait
            # on their cast semaphores and stream as load data arrives.
            for p0_ in range(npair):
                cast_pair(0, p0_)
                enqueue_transposes(0, p0_)
            drain_transposes(len(trans_q))

            # ---- deferred-emission machinery ----
            pending_scores = deque()  # (b, g2, hoc, ready_slot)
            pending_tail = deque()    # (kind, b, g2, ready_slot)
            hTs = {}
            score_banks = {}
            tails = {}

            def emit_scores(t):
                while pending_scores and pending_scores[0][3] <= t:
                    b_, g2, hoc, _ = pending_scores.popleft()
                    if hoc == 0:
                        score_banks[(b_, g2)] = pssc.tile([128, H], f32, tag="sc", name="ssum")
                    ssum = score_banks[(b_, g2)]
                    for pp in range(2):
                        hT = hTs.pop((b_, 2 * g2 + pp, hoc))
                        for bi in range(2):
                            row = 32 * (2 * pp + bi)
                            nc.tensor.matmul(
                                ssum[row:row + 32, :], Wo32_sb[:, hoc, :],
                                hT[:, 512 * bi:512 * (bi + 1)],
                                start=(hoc == 0), stop=(hoc == HC - 1),
                                tile_position=(0, row),
                            )
                    if hoc == HC - 1:
                        pending_tail.append(("s97", b_, g2, t + 1))

            def emit_tail(t):
                if not (pending_tail and pending_tail[0][3] <= t):
                    return
                kind, b_, g2, _ = pending_tail.popleft()
                if kind == "s97":
                    ssum = score_banks.pop((b_, g2))
                    nc.vector.tensor_copy(s97[g2][:], ssum[0:97, :])
                    if g2 == 1:
                        pending_tail.append(("taila", b_, None, t + 1))
                elif kind == "taila":
                    pse = pstl.tile([128, HC, 8], f32, tag="tl", name="pse")
                    for j in range(HC):
                        for g2_ in range(2):
                            nc.tensor.matmul(
                                pse[:, j, :], s97[g2_][:, 128 * j:128 * (j + 1)],
                                sel_sb[:, g2_, :], start=(g2_ == 0), stop=(g2_ == 1),
                            )
                    escT = smpool.tile([128, HC, 8], bf16, tag="escT", name="escT")
                    eacc = smpool.tile([128, 1], f32, tag="eacc", name="eacc")
                    nc.scalar.activation(escT[:], pse[:], AF.Exp, accum_out=eacc[:])
                    tails[b_] = (escT, eacc)
                    pending_tail.append(("tailb", b_, None, t + 1))
                elif kind == "tailb":
                    escT, eacc = tails.pop(b_)
                    tot_ps = pstl.tile([1, 1], f32, tag="tl", name="totps")
                    nc.tensor.matmul(tot_ps[:], eacc[:], ones128f[:],
                                     start=True, stop=True)
                    rec = smpool.tile([1, 1], f32, tag="rec", name="rec")
                    nc.vector.reciprocal(rec[:], tot_ps[:])
                    ctx_ps = pstl.tile([128, H], f32, tag="tl", name="ctxps")
                    for blk in range(nblk):
                        for j in range(HC):
                            nc.tensor.matmul(
                                ctx_ps[32 * j:32 * j + 1, :],
                                escT[:, j, blk:blk + 1],
                                vnats[(b_, blk // 4)][:, blk % 4, j, :],
                                start=(blk == 0), stop=(blk == nblk - 1),
                                tile_position=(0, 32 * j),
                            )
                    for j in range(HC):
                        nc.vector.tensor_copy(
                            ctx97[32 * j:32 * j + 1, :], ctx_ps[32 * j:32 * j + 1, :],
                        )
                    cs_ps = pstl.tile([1, H], f32, tag="tl", name="csps")
                    nc.tensor.matmul(cs_ps[:], ones128[0:97, :], ctx97[:],
                                     start=True, stop=True)
                    outrow = smpool.tile([1, H], f32, tag="outrow", name="outrow")
                    nc.vector.tensor_scalar_mul(outrow[:], cs_ps[:], rec[:])
                    nc.sync.dma_start(out=out_d[b_:b_ + 1, :], in_=outrow[:])
                    del vnats[(b_, 0)], vnats[(b_, 1)]

            # value pipeline calendar during batch b: casts for b+1 (loads
            # landed a batch ago, so DVE never blocks), transposes for b+1
            # enqueued behind each cast and drained a few per slot, loads
            # for b+2 (free-running pure HBM stream).
            def cal_events(b, p, hoc):
                nb, nnb = b + 1, b + 2
                k = (p, hoc)
                if nb < b_per_core:
                    if k == (0, 0):
                        cast_pair(nb, 0)
                        enqueue_transposes(nb, 0)
                    elif k == (0, 1):
                        cast_pair(nb, 1)
                        enqueue_transposes(nb, 1)
                    elif k == (1, 0):
                        cast_pair(nb, 2)
                        enqueue_transposes(nb, 2)
                    elif k == (1, 1):
                        cast_pair(nb, 3)
                        enqueue_transposes(nb, 3)
                if nnb < b_per_core:
                    if k == (2, 0):
                        load_group(nnb, 0)
                    elif k == (3, 0):
                        load_group(nnb, 1)

            # ---------------- main pipeline ----------------
            t = 0
            for b in range(b_per_core):
                for p in range(npair):
                    vt8 = vT8s.pop((b, p))
                    vt8f = vt8[:].bitcast(fp8)   # [128, 16, 256]
                    for hoc in range(HC):
                        emit_scores(t)
                        emit_tail(t)
                        cal_events(b, p, hoc)
                        drain_transposes(6)
                        zp = psz.tile([128, 1024], f32, tag="z", name="zp")
                        for g in range(2):
                            lhsT = Wv8_sb[:, g, :, 128 * hoc:128 * (hoc + 1)]
                            for bi in range(2):
                                c0 = 4 * (2 * bi + g)
                                rhs = vt8f[:, c0:c0 + 4].rearrange(
                                    "p j (s i) -> p i (j s)", i=2
                                )
                                nc.tensor.matmul(
                                    zp[:, 512 * bi:512 * (bi + 1)], lhsT, rhs,
                                    start=(g == 0), stop=(g == 1), perf_mode=DR,
                                )
                        hT = hpool.tile([128, 1024], bf16, tag="ht", name="hT")
                        nc.scalar.activation(
                            hT[:], zp[:], AF.Tanh,
                            bias=qcT[:, hoc, b:b + 1], scale=1.0 / WV_SCALE,
                        )
                        hTs[(b, p, hoc)] = hT
                        if p % 2 == 1:
                            pending_scores.append((b, p // 2, hoc, t + 2))
                        t += 1

            # drain
            while pending_scores or pending_tail:
                emit_scores(t)
                emit_tail(t)
                t += 1

    nc.compile()
    return nc


def make_in_maps(query, value, Wq, bq, Wv, bv, Wo, bo):
    """Host-side prep: shard + precompute small tensors. query [1,32,512]."""
    import ml_dtypes

    query = np.asarray(query, dtype=np.float32)
    value = np.asarray(value, dtype=np.float32)
    Wq = np.asarray(Wq, dtype=np.float32)
    bq = np.asarray(bq, dtype=np.float32)
    Wv = np.asarray(Wv, dtype=np.float32)
    bv = np.asarray(bv, dtype=np.float32)
    Wo = np.asarray(Wo, dtype=np.float32)

    # qcomb[b, h] = q[b] @ Wq + bq + bv
    qcomb = query[0] @ Wq + bq + bv                    # [32, 512]
    # Wv8[p, g, i, ho] = fp8(32 * Wv[256g + 2p + i, ho])
    wv_s = (WV_SCALE * Wv).reshape(2, 128, 2, H)       # [g, p, i, ho]
    wv8 = np.ascontiguousarray(
        wv_s.transpose(1, 0, 2, 3)).astype(ml_dtypes.float8_e4m3)  # [128,2,2,H]
    # Wo strips: [128, hc, 32], col 0 = Wo chunk, rest 0
    wo32 = np.zeros((128, HC, 32), np.float32)
    wo32[:, :, 0] = Wo[:, 0].reshape(HC, 128).T
    wo32 = wo32.astype(ml_dtypes.bfloat16)
    # selectors [2, 97, 8]
    sel = np.zeros((2, 97, 8), np.float32)
    for g2 in range(2):
        for a in range(4):
            sel[g2, 32 * a, g2 * 4 + a] = 1.0

    in_maps = []
    for i in range(N_CORES):
        sl = slice(B * i, B * (i + 1))
        qcT = np.ascontiguousarray(
            qcomb[sl].reshape(B, HC, 128).transpose(2, 1, 0))  # [128, HC, B]
        in_maps.append({
            "value": np.ascontiguousarray(value[sl]),
            "wv8": wv8,
            "qcombT": qcT,
            "wo32": wo32,
            "sel": sel,
        })
    return in_maps


def kernel(**inputs):
    from concourse.bass_utils import run_bass_kernel_spmd

    key = "full"
    if key not in _cache:
        _cache[key] = build_nc()
    nc = _cache[key]

    in_maps = make_in_maps(
        inputs["query"], inputs["value"], inputs["Wq"], inputs["bq"],
        inputs["Wv"], inputs["bv"], inputs["Wo"], inputs["bo"],
    )
    res = run_bass_kernel_spmd(nc, in_maps, core_ids=list(range(N_CORES)))
    out = np.concatenate([res.results[i]["out"] for i in range(N_CORES)], axis=0)
    return out[:, None, :].astype(np.float32)  # [32, 1, 512]


# revision 25
# speedup vs baseline: 1.4655x; 1.0838x over previous
"""AdditiveAttention (Bahdanau) distributed Bass kernel for 8 TRN2 NeuronCores.

Computation (per batch b):
    qc[b,:]   = query[b] @ Wq + bq + bv                       # [512]  (HOST)
    z[b,s,:]  = value[b,s] @ Wv + qc[b]                       # pre-tanh
    score     = tanh(z) @ Wo          (+bo dropped: cancels in softmax)
    align     = softmax(score)        (no max-sub: |score| <= ~23)
    out[b,:]  = align @ value[b]

Sharding: data-parallel over batch, 4 batches per core, weights replicated.

v3 design (per core: B=4 batches, SEQ=4096, H=512):
  - HOST prep: qcombT (q-projection, transposed), Wo strip-padded to 32
    cols, selector matrices, and Wv pre-scaled x32 + packed to fp8e4 in
    the DoubleRow interleave h = 256*g + 2*p + i.
  - value: SWDGE cast-DMA f32->bf16 natural group tiles (context path),
    then DVE cast bf16->fp8 natural, then ONE xbar DMA-transpose per
    2-block pair of the fp8 data VIEWED AS bf16 pairs -> vT8 where each
    16-bit unit holds (h, h+1) adjacent fp8 values: exactly DoubleRow's
    moving-operand format. Transposed DMA bytes halve vs bf16 (8.4 MB).
  - main z matmuls: fp8 DoubleRow, K=256/pass -> 4 MMs of N=512 per
    (pair, hoc) accumulating a [128,1024] 2-bank PSUM tile (64 MMs/batch
    vs 128 bf16 MMs in v2).
  - tanh on ACT, N=1024 per instruction (vs 512), scale=1/32 undoes the
    Wv prescale, bias=qcombT adds the query projection.
  - scores: RAW (pre-exp) score strips accumulated in one PSUM bank per
    4-block group, 4-way col-tiled (M=32 strips so all 97 partitions are
    written), then ONE DVE copy -> s97, selector MMs transpose raw
    scores, and ONE exp per batch [128,32] with accum_out giving the
    softmax total for free. (v2 spent 23us/core on [1,512] exp calls.)
  - context: 32 accumulating MMs escT^T @ v_nat -> [1,512] rows, scale
    by 1/total, store via sync-queue DMA.
  - pipeline: slot calendar (b,pair,hoc); scores trail tanh by 2 slots;
    tails and next-batch value pipeline interleave into the z stream.
"""

import numpy as np

N_CORES = 8
BATCH_TOTAL = 32
B = BATCH_TOTAL // N_CORES  # batches per core
SEQ = 4096
H = 512
HC = H // 128   # 4 hidden chunks
NBLK = SEQ // 512   # 8 seq blocks per batch
NPAIR = NBLK // 2
WV_SCALE = 32.0

_cache = {}


def build_nc(b_per_core=B, seq=SEQ):
    from collections import deque

    import concourse.bass as bass
    import concourse.mybir as mybir
    import concourse.tile as tile
    from concourse import bacc
    from concourse.masks import make_identity

    f32 = mybir.dt.float32
    bf16 = mybir.dt.bfloat16
    fp8 = mybir.dt.float8e4
    AF = mybir.ActivationFunctionType
    DR = mybir.MatmulPerfMode.DoubleRow

    nblk = seq // 512
    npair = nblk // 2

    nc = bacc.Bacc("TRN2", target_bir_lowering=False, debug=False)

    val_d = nc.dram_tensor("value", [b_per_core, seq, H], f32, kind="ExternalInput").ap()
    wv8_d = nc.dram_tensor("wv8", [128, 2, 2, H], fp8, kind="ExternalInput").ap()
    qct_d = nc.dram_tensor("qcombT", [128, HC, b_per_core], f32, kind="ExternalInput").ap()
    wo32_d = nc.dram_tensor("wo32", [128, HC, 32], bf16, kind="ExternalInput").ap()
    sel_d = nc.dram_tensor("sel", [2, 98, 8], f32, kind="ExternalInput").ap()
    u32_d = nc.dram_tensor("u32", [b_per_core, 128, 4, 32], fp8, kind="ExternalInput").ap()
    out_d = nc.dram_tensor("out", [b_per_core, H], f32, kind="ExternalOutput").ap()

    # s = (g2*4 + blk)*512 + p*4 + j
    val_v = val_d.rearrange(
        "b (g blk p j) h -> b g p blk j h", g=2, blk=4, p=128, j=4
    )

    with tile.TileContext(nc) as tc:
        with (
            tc.tile_pool(name="weights", bufs=1) as wpool,
            tc.tile_pool(name="vnat", bufs=6) as vpool,
            tc.tile_pool(name="v8", bufs=3) as v8pool,
            tc.tile_pool(name="vt8", bufs=7) as tpool,
            tc.tile_pool(name="ht", bufs=9) as hpool,
            tc.tile_pool(name="small", bufs=6) as smpool,
            tc.tile_pool(name="psum_z", bufs=2, space="PSUM") as psz,
            tc.tile_pool(name="psum_tr", bufs=2, space="PSUM") as ptr,
            tc.tile_pool(name="psum_sc", bufs=1, space="PSUM") as pssc,
            tc.tile_pool(name="psum_tl", bufs=1, space="PSUM") as pstl,
        ):
            # ---- persistent SBUF residents ----
            Wv8_sb = wpool.tile([128, 2, 2, H], fp8)
            qcT = wpool.tile([128, HC, b_per_core], f32)
            Wo32_sb = wpool.tile([128, HC, 32], bf16)
            sel_sb = wpool.tile([98, 2, 8], f32)
            u32_sb = wpool.tile([128, b_per_core, 4, 32], fp8)
            ones128 = wpool.tile([128, 1], bf16)
            ones128f = wpool.tile([128, 1], f32)
            s97 = [wpool.tile([98, H], f32, name=f"s97_{g}") for g in range(2)]
            ctx97 = wpool.tile([97, H], bf16)
            warm = wpool.tile([128, H], bf16)
            prew_out = wpool.tile([1, 32], f32)
            ident = wpool.tile([128, 128], bf16)

            # ---- value pipeline ----
            vnats = {}   # (b, g2) -> bf16 natural tile [128, 4, 4, 512]
            v8s = {}     # (b, g2) -> fp8 natural tile  [128, 4, 2, 4, 256]
            vT8s = {}    # (b, p)  -> packed transpose  [128, 16, 128] bf16 units

            def load_group(b, g2):
                vt = vpool.tile([128, 4, 4, H], bf16, tag="vnat", name="vg")
                nc.gpsimd.dma_start(out=vt[:, 0:2], in_=val_v[b, g2, :, 0:2])
                nc.gpsimd.dma_start(out=vt[:, 2:4], in_=val_v[b, g2, :, 2:4])
                vnats[(b, g2)] = vt

            def cast_pair(b, pair):
                g2, q = pair // 2, pair % 2
                if (b, g2) not in v8s:
                    v8s[(b, g2)] = v8pool.tile(
                        [128, 4, 2, 4, 256], fp8, tag="v8", name="v8g")
                v8 = v8s[(b, g2)]
                vt = vnats[(b, g2)]
                for g in range(2):
                    nc.vector.tensor_copy(
                        v8[:, 2 * q:2 * q + 2, g],
                        vt[:, 2 * q:2 * q + 2, :, 256 * g:256 * (g + 1)])

            # PE-side transpose of the packed units: each [128,128] chunk of
            # the fp8-pair data (viewed as bf16 units) goes through matmul
            # transpose-mode into a bf16 PSUM bank (8 chunks per bank), then
            # one DVE copy lands it in vT8. No DMA-transposes at all: the
            # Tile framework serializes those against every other DMA (HW
            # deadlock guard), which lock-stepped loads and transposes into
            # a ~12us alternation in the v3a trace.
            def transpose_chunk(b, p, c, ps_tr):
                if (b, p) not in vT8s:
                    vT8s[(b, p)] = tpool.tile([128, 16, 128], bf16, tag="vt8",
                                              name="vt8")
                g2, q = p // 2, p % 2
                src = v8s[(b, g2)][:].bitcast(bf16).rearrange(
                    "p a b c d -> p (a b c d)")
                lo = q * 2048 + c * 128
                nc.tensor.matmul(ps_tr[:, c % 8, :], src[:, lo:lo + 128],
                                 ident[:], start=True, stop=True,
                                 is_transpose=True)

            def transpose_flush(b, p, c8, ps_tr):
                # copy chunks [c8, c8+8) of pair p from psum to vT8
                nc.vector.tensor_copy(vT8s[(b, p)][:, c8:c8 + 8, :], ps_tr[:])

            # transpose work queue: ("t", b, p, c) chunk transposes and
            # ("f", b, p, c8) psum->vT8 flushes, drained a few per slot.
            trans_q = deque()
            cur_ps = [None]

            def enqueue_transposes(b, p):
                for c in range(16):
                    trans_q.append(("t", b, p, c))
                    if c % 8 == 7:
                        trans_q.append(("f", b, p, c - 7))

            def drain_transposes(n):
                for _ in range(n):
                    if not trans_q:
                        return
                    it = trans_q.popleft()
                    if it[0] == "t":
                        _, b_, p_, c_ = it
                        if c_ % 8 == 0:
                            cur_ps[0] = ptr.tile([128, 8, 128], bf16,
                                                 tag="tr", name="pstr")
                        transpose_chunk(b_, p_, c_, cur_ps[0])
                    else:
                        _, b_, p_, c8 = it
                        transpose_flush(b_, p_, c8, cur_ps[0])

            # prologue — value-load triggers FIRST on the SWDGE queue (the
            # critical path; pure HBM loads now, so they free-run).
            load_group(0, 0)
            nc.gpsimd.memset(warm[:], 0.0)
            # ACT table prewarm: exp+tanh live in one set; load it while
            # the first value tiles stream in, not at first real tanh.
            nc.scalar.activation(prew_out[:], warm[0:1, 0:32], AF.Exp)
            nc.scalar.activation(prew_out[:], warm[0:1, 0:32], AF.Tanh)
            # weights ride the scalar (ACT) HWDGE queue: off the SWDGE
            # value-load critical path, done within a few us.
            nc.scalar.dma_start(out=Wv8_sb[:], in_=wv8_d)
            nc.scalar.dma_start(out=qcT[:], in_=qct_d)
            nc.scalar.dma_start(out=Wo32_sb[:], in_=wo32_d)
            nc.scalar.dma_start(out=sel_sb[:], in_=sel_d.rearrange("g p r -> p g r"))
            nc.scalar.dma_start(out=u32_sb[:], in_=u32_d.rearrange("b p k c -> p b k c"))
            load_group(0, 1)
            if b_per_core > 1:
                load_group(1, 0)
                load_group(1, 1)
            nc.gpsimd.memset(ones128[:], 1.0)
            nc.gpsimd.memset(ones128f[:], 1.0)
            nc.gpsimd.memset(ctx97[:], 0.0)
            make_identity(nc, ident[:])

            # PE warmup: fill the HAM activity window while value loads run.
            ps_warm = pstl.tile([128, H], f32, tag="tl", name="pswarm")
            for _ in range(20):
                nc.tensor.matmul(ps_warm[:], warm[:, 0:128], warm[:],
                                 start=True, stop=True)

            # batch-0 casts + transposes up front, PER PAIR so each pair's
            # psum->vT8 flush sits right behind its own cast in the DVE
            # queue (flushes behind all 8 data-gated casts cost 28us of PE
            # idle in the v3c trace).
            for p0_ in range(npair):
                cast_pair(0, p0_)
                enqueue_transposes(0, p0_)
                drain_transposes(len(trans_q))

            # ---- deferred-emission machinery ----
            pending_scores = deque()  # (b, g2, hoc, ready_slot)
            pending_tail = deque()    # (kind, b, g2, ready_slot)
            hTs = {}
            score_banks = {}
            tails = {}
            vt8fs = {}                # (b, p) -> fp8 AP view, for corrections

            def emit_scores(t):
                while pending_scores and pending_scores[0][3] <= t:
                    b_, g2, hoc, _ = pending_scores.popleft()
                    if hoc == 0:
                        score_banks[(b_, g2)] = pssc.tile([128, H], f32, tag="sc", name="ssum")
                    ssum = score_banks[(b_, g2)]
                    for pp in range(2):
                        hT = hTs.pop((b_, 2 * g2 + pp, hoc))
                        for bi in range(2):
                            row = 32 * (2 * pp + bi)
                            nc.tensor.matmul(
                                ssum[row:row + 32, :], Wo32_sb[:, hoc, :],
                                hT[:, 512 * bi:512 * (bi + 1)],
                                start=(hoc == 0), stop=False,
                                tile_position=(0, row),
                            )
                    if hoc == HC - 1:
                        # Wv-quantization correction rows: score strips left
                        # rows 32a+1 zero; accumulate corr = v8 . u there
                        # (1024x-scaled, in column 1 of an M=32 fp8 strip, so
                        # the MM shape matches the proven score strips); the
                        # selector subtracts it with coefficient 1/1024.
                        for g in range(2):
                            for i_ in range(2):
                                lhsT = u32_sb[:, b_, 2 * g + i_, :]
                                for a in range(4):
                                    pp, bi = a // 2, a % 2
                                    vt8f = vt8fs[(b_, 2 * g2 + pp)]
                                    c0 = 4 * (2 * bi + g)
                                    rhs = vt8f[:, c0:c0 + 4].rearrange(
                                        "p j (s i) -> p i (j s)", i=2)[:, i_, :]
                                    nc.tensor.matmul(
                                        ssum[32 * a:32 * a + 32, :], lhsT, rhs,
                                        start=False, stop=(g == 1 and i_ == 1),
                                        tile_position=(0, 32 * a),
                                        skip_group_check=True,
                                    )
                        pending_tail.append(("s97", b_, g2, t + 1))

            def emit_tail(t):
                if not (pending_tail and pending_tail[0][3] <= t):
                    return
                kind, b_, g2, _ = pending_tail.popleft()
                if kind == "s97":
                    ssum = score_banks.pop((b_, g2))
                    nc.vector.tensor_copy(s97[g2][:], ssum[0:98, :])
                    del vt8fs[(b_, 2 * g2)], vt8fs[(b_, 2 * g2 + 1)]
                    if g2 == 1:
                        pending_tail.append(("taila", b_, None, t + 1))
                elif kind == "taila":
                    pse = pstl.tile([128, HC, 8], f32, tag="tl", name="pse")
                    for j in range(HC):
                        for g2_ in range(2):
                            nc.tensor.matmul(
                                pse[:, j, :], s97[g2_][:, 128 * j:128 * (j + 1)],
                                sel_sb[:, g2_, :], start=(g2_ == 0), stop=(g2_ == 1),
                            )
                    escT = smpool.tile([128, HC, 8], bf16, tag="escT", name="escT")
                    eacc = smpool.tile([128, 1], f32, tag="eacc", name="eacc")
                    nc.scalar.activation(escT[:], pse[:], AF.Exp, accum_out=eacc[:])
                    tails[b_] = (escT, eacc)
                    pending_tail.append(("tailb", b_, None, t + 1))
                elif kind == "tailb":
                    escT, eacc = tails.pop(b_)
                    tot_ps = pstl.tile([1, 1], f32, tag="tl", name="totps")
                    nc.tensor.matmul(tot_ps[:], eacc[:], ones128f[:],
                                     start=True, stop=True)
                    rec = smpool.tile([1, 1], f32, tag="rec", name="rec")
                    nc.vector.reciprocal(rec[:], tot_ps[:])
                    ctx_ps = pstl.tile([128, H], f32, tag="tl", name="ctxps")
                    for blk in range(nblk):
                        for j in range(HC):
                            nc.tensor.matmul(
                                ctx_ps[32 * j:32 * j + 1, :],
                                escT[:, j, blk:blk + 1],
                                vnats[(b_, blk // 4)][:, blk % 4, j, :],
                                start=(blk == 0), stop=(blk == nblk - 1),
                                tile_position=(0, 32 * j),
                            )
                    for j in range(HC):
                        nc.vector.tensor_copy(
                            ctx97[32 * j:32 * j + 1, :], ctx_ps[32 * j:32 * j + 1, :],
                        )
                    cs_ps = pstl.tile([1, H], f32, tag="tl", name="csps")
                    nc.tensor.matmul(cs_ps[:], ones128[0:97, :], ctx97[:],
                                     start=True, stop=True)
                    outrow = smpool.tile([1, H], f32, tag="outrow", name="outrow")
                    nc.vector.tensor_scalar_mul(outrow[:], cs_ps[:], rec[:])
                    nc.sync.dma_start(out=out_d[b_:b_ + 1, :], in_=outrow[:])
                    del vnats[(b_, 0)], vnats[(b_, 1)]

            # value pipeline calendar during batch b: casts for b+1 (loads
            # landed a batch ago, so DVE never blocks), transposes for b+1
            # enqueued behind each cast and drained a few per slot, loads
            # for b+2 (free-running pure HBM stream).
            def cal_events(b, p, hoc):
                nb, nnb = b + 1, b + 2
                k = (p, hoc)
                if nb < b_per_core:
                    if k == (0, 0):
                        cast_pair(nb, 0)
                        enqueue_transposes(nb, 0)
                    elif k == (0, 1):
                        cast_pair(nb, 1)
                        enqueue_transposes(nb, 1)
                    elif k == (1, 0):
                        cast_pair(nb, 2)
                        enqueue_transposes(nb, 2)
                    elif k == (1, 1):
                        cast_pair(nb, 3)
                        enqueue_transposes(nb, 3)
                if nnb < b_per_core:
                    if k == (2, 0):
                        load_group(nnb, 0)
                    elif k == (3, 0):
                        load_group(nnb, 1)

            # ---------------- main pipeline ----------------
            t = 0
            for b in range(b_per_core):
                for p in range(npair):
                    vt8 = vT8s.pop((b, p))
                    vt8f = vt8[:].bitcast(fp8)   # [128, 16, 256]
                    vt8fs[(b, p)] = vt8f
                    for hoc in range(HC):
                        emit_scores(t)
                        emit_tail(t)
                        cal_events(b, p, hoc)
                        drain_transposes(6)
                        zp = psz.tile([128, 1024], f32, tag="z", name="zp")
                        for g in range(2):
                            lhsT = Wv8_sb[:, g, :, 128 * hoc:128 * (hoc + 1)]
                            for bi in range(2):
                                c0 = 4 * (2 * bi + g)
                                rhs = vt8f[:, c0:c0 + 4].rearrange(
                                    "p j (s i) -> p i (j s)", i=2
                                )
                                nc.tensor.matmul(
                                    zp[:, 512 * bi:512 * (bi + 1)], lhsT, rhs,
                                    start=(g == 0), stop=(g == 1), perf_mode=DR,
                                )
                        hT = hpool.tile([128, 1024], bf16, tag="ht", name="hT")
                        nc.scalar.activation(
                            hT[:], zp[:], AF.Tanh,
                            bias=qcT[:, hoc, b:b + 1], scale=1.0 / WV_SCALE,
                        )
                        hTs[(b, p, hoc)] = hT
                        if p % 2 == 1:
                            pending_scores.append((b, p // 2, hoc, t + 2))
                        t += 1

            # drain
            while pending_scores or pending_tail:
                emit_scores(t)
                emit_tail(t)
                t += 1

    nc.compile()
    return nc


def make_in_maps(query, value, Wq, bq, Wv, bv, Wo, bo):
    """Host-side prep: shard + precompute small tensors. query [1,32,512]."""
    import ml_dtypes

    query = np.asarray(query, dtype=np.float32)
    value = np.asarray(value, dtype=np.float32)
    Wq = np.asarray(Wq, dtype=np.float32)
    bq = np.asarray(bq, dtype=np.float32)
    Wv = np.asarray(Wv, dtype=np.float32)
    bv = np.asarray(bv, dtype=np.float32)
    Wo = np.asarray(Wo, dtype=np.float32)

    # qcomb[b, h] = q[b] @ Wq + bq + bv
    qcomb = query[0] @ Wq + bq + bv                    # [32, 512]
    # Wv8[p, g, i, ho] = fp8(32 * Wv[256g + 2p + i, ho])
    wv_s = (WV_SCALE * Wv).reshape(2, 128, 2, H)       # [g, p, i, ho]
    wv8 = np.ascontiguousarray(
        wv_s.transpose(1, 0, 2, 3)).astype(ml_dtypes.float8_e4m3)  # [128,2,2,H]
    # Wo strips: [128, hc, 32], col 0 = Wo chunk, rest 0
    wo32 = np.zeros((128, HC, 32), np.float32)
    wo32[:, :, 0] = Wo[:, 0].reshape(HC, 128).T
    wo32 = wo32.astype(ml_dtypes.bfloat16)
    # selectors [2, 98, 8]: row 32a picks block score, row 32a+1 subtracts
    # the 1024x-scaled Wv-quantization correction
    sel = np.zeros((2, 98, 8), np.float32)
    for g2 in range(2):
        for a in range(4):
            sel[g2, 32 * a, g2 * 4 + a] = 1.0
            sel[g2, 32 * a + 1, g2 * 4 + a] = -1.0 / 1024.0

    # Wv-quantization score-correction weights: the fp8 weight error eW is
    # shared across all seq positions, creating a systematic score shift
    # ~ v_s . U_b with U_b[h] = sum_ho E[tanh'(z_bho)] * eW[h,ho] * Wo[ho].
    eW = wv8.astype(np.float32).transpose(1, 0, 2, 3).reshape(H, H) / WV_SCALE - Wv
    sig = np.sqrt((Wv ** 2).sum(0))                    # [512] std of v@Wv col
    gh_x, gh_w = np.polynomial.hermite_e.hermegauss(21)
    gh_w = gh_w / gh_w.sum()
    zz = qcomb[:, None, :] + sig[None, None, :] * gh_x[None, :, None]
    C = (gh_w[None, :, None] * (1.0 - np.tanh(zz) ** 2)).sum(1)  # [32, 512]
    Wo_b = wo32[:, :, 0].astype(np.float32).T.reshape(H)         # bf16-rounded Wo
    U = np.einsum('bk,hk,k->bh', C, eW, Wo_b)          # [32, 512]
    # u32[b, p, 2g+i, col] strip weights: col 1 = fp8(1024*U[b, 256g+2p+i])
    u8v = (1024.0 * U).reshape(32, 2, 128, 2).transpose(0, 2, 1, 3)  # [b,p,g,i]
    u32_full = np.zeros((32, 128, 4, 32), np.float32)
    u32_full[:, :, 0, 1] = u8v[:, :, 0, 0]
    u32_full[:, :, 1, 1] = u8v[:, :, 0, 1]
    u32_full[:, :, 2, 1] = u8v[:, :, 1, 0]
    u32_full[:, :, 3, 1] = u8v[:, :, 1, 1]
    u32_full = u32_full.astype(ml_dtypes.float8_e4m3)

    in_maps = []
    for i in range(N_CORES):
        sl = slice(B * i, B * (i + 1))
        qcT = np.ascontiguousarray(
            qcomb[sl].reshape(B, HC, 128).transpose(2, 1, 0))  # [128, HC, B]
        in_maps.append({
            "value": np.ascontiguousarray(value[sl]),
            "wv8": wv8,
            "qcombT": qcT,
            "wo32": wo32,
            "sel": sel,
            "u32": np.ascontiguousarray(u32_full[sl]),
        })
    return in_maps


def kernel(**inputs):
    from concourse.bass_utils import run_bass_kernel_spmd

    key = "full"
    if key not in _cache:
        _cache[key] = build_nc()
    nc = _cache[key]

    in_maps = make_in_maps(
        inputs["query"], inputs["value"], inputs["Wq"], inputs["bq"],
        inputs["Wv"], inputs["bv"], inputs["Wo"], inputs["bo"],
    )
    res = run_bass_kernel_spmd(nc, in_maps, core_ids=list(range(N_CORES)))
    out = np.concatenate([res.results[i]["out"] for i in range(N_CORES)], axis=0)
    return out[:, None, :].astype(np.float32)  # [32, 1, 512]
